# revision 12
# baseline (speedup 1.0000x reference)
"""Trainium2 Bass kernel for nn_AttentionBlock (GroupNorm + windowed MHA + proj + residual).

Contract: kernel(**inputs) takes FULL unsharded inputs (as from reference.setup_inputs())
and returns the FULL output [1, 256, 96, 96] float32.

Sharding: sequence-parallel over query positions across 8 cores. Each core gets a
uniform slice of each of the 3 reference attention windows:
  W0: q[512i   : 512(i+1)]    attends kv[0    : 6144]
  W1: q[4096+512i : ...]      attends kv[2048 : 9216]
  W2: q[8192+128i : ...]      attends kv[6144 : 9216]
All 4 heads for those queries are computed locally, so the output projection and
residual are local too. Every core redundantly computes GroupNorm stats and the
full-sequence K/V (needed since every core's windows span nearly the full sequence).

On-chip dataflow ("transposed" layout, channels on partitions):
  x [256, 9216]  -> bn_stats -> group stats via small PE matmuls -> per-channel a,b
  xn = a*x + b   (DVE, streamed)
  kT [2heads*64, 9216] = Wk-slice.T-matmuls (per pass of 2 heads)
  v  [9216, 2heads*65] (65th col = ones, so PV matmul also produces softmax sums)
  q  [2heads*64, 1152]
  S^T[keys, q] = kT-chunk as lhsT, qT as rhs; exp on ScalarE (PSUM->SBUF, batched
  3 key-chunks per ACTIVATE); PV: lhsT=[v|1], rhs=exp(S^T) accumulating O^T[65, q];
  normalize by broadcast reciprocal row (DRAM-bounce partition broadcast);
  proj: projT-matmuls over assembled attn^T [256, 1152] + bias (via rank-1 matmul)
  + residual, DMA out.
"""

import numpy as np

import concourse.bass as bass
import concourse.tile as tile
from concourse import mybir
from concourse.vector_clock import ScopedClock, VectorClock

F32 = mybir.dt.float32
AF = mybir.ActivationFunctionType
ALU = mybir.AluOpType

C = 256
SEQ = 9216
NCORES = 8
HEADS = 4
D = 64
EPS = 1e-5
SCALE = 0.125  # 1/sqrt(64)
NQC = 1152  # queries per core
ST = 512  # seq tile for qkv streaming
NST = SEQ // ST  # 18
# windows: (q_off, q_len, key_chunk0, n_key_chunks)
WINDOWS = [(0, 512, 0, 48), (512, 512, 16, 56), (1024, 128, 48, 24)]
SCHUNK = 3  # key-chunks batched per exp ACTIVATE (3 PSUM banks)


def _patch_tile_drain():
    """This container's walrus rejects >1 sem wait on one sync CTRL instruction
    ("Too many sync wait commands"). Split the TileContext-exit drain's waits
    into one-wait-per-nop instructions."""
    if getattr(tile.TileContext, "_drain_split_patched", False):
        return

    def _drain_and_barrier(self, tick_clock, wait_clock):
        vc = tick_clock.global_clock
        n = len(vc)
        for p in range(n):
            t = vc[p]
            if t <= 0:
                continue
            single = VectorClock([t if i == p else 0 for i in range(n)])
            inst = self.nc.sync.nop(nofuse=True, hint="drain_split_wait")
            wait_clock.add_sem_waits(inst.ins, ScopedClock({None: single}))
        self.nc.sync.drain()
        self.nc.all_engine_barrier()
        assert self.sems is not None
        popped = self.nc._tile_sem_poison_stack.pop()
        assert popped is self._sem_poison
        self.nc.clear_and_free_semaphores(list(self.sems.allocated().values()))
        self.nc.all_engine_barrier()

    tile.TileContext._drain_and_barrier = _drain_and_barrier
    tile.TileContext._drain_split_patched = True


def _patch_to_json_split_waits():
    """This walrus build rejects instructions carrying more than one sem-wait
    ("Too many sync wait commands"). Post-process the BIR JSON: keep one wait
    on each instruction and move extras onto same-engine NoOps inserted just
    before it (identical sync semantics: the engine blocks on the nops first)."""
    if getattr(bass.Bass, "_split_waits_patched", False):
        return
    import json as _json

    orig = bass.Bass.to_json_bytes

    def to_json_bytes(self):
        d = _json.loads(orig(self))
        for fn in d["functions"]:
            for blk in fn["blocks"]:
                out = []
                changed = False
                for ins in blk["instructions"]:
                    si = ins.get("sync_info")
                    waits = (si or {}).get("on_wait") or []
                    if len(waits) > 1:
                        changed = True
                        for k, w in enumerate(waits[:-1]):
                            out.append({
                                "debug": ins.get("debug", 0),
                                "engine": ins["engine"],
                                "ins": [],
                                "name": f"{ins['name']}-w{k}",
                                "opcode": "NoOp",
                                "outs": [],
                                "sync_info": {"on_update": [], "on_wait": [w]},
                                "text_hint": "split_wait",
                            })
                        si["on_wait"] = [waits[-1]]
                    out.append(ins)
                if changed:
                    blk["instructions"] = out
        return _json.dumps(d).encode()

    bass.Bass.to_json_bytes = to_json_bytes
    bass.Bass._split_waits_patched = True


def _bcast_part(ap, n):
    """[1, m] AP -> [n, m] AP broadcasting along partitions (step 0)."""
    apl = ap.ap
    assert apl[0][1] == 1, apl
    return bass.AP(tensor=ap.tensor, offset=ap.offset, ap=[[0, n]] + [list(d) for d in apl[1:]])


def build_nc(phase=99):
    nc = bass.Bass()

    x_d = nc.dram_tensor("x", [C, SEQ], F32, kind="ExternalInput")
    xq_d = nc.dram_tensor("xq", [C, NQC], F32, kind="ExternalInput")
    wT_d = nc.dram_tensor("wT", [C, 3 * C], F32, kind="ExternalInput")
    projT_d = nc.dram_tensor("projT", [C, C], F32, kind="ExternalInput")
    pvec_d = nc.dram_tensor("pvec", [128, 8], F32, kind="ExternalInput")
    projbr_d = nc.dram_tensor("projbr", [1, C], F32, kind="ExternalInput")
    G_d = nc.dram_tensor("G", [128, 16], F32, kind="ExternalInput")
    GT_d = nc.dram_tensor("GT", [16, 128], F32, kind="ExternalInput")
    out_d = nc.dram_tensor("out", [C, NQC], F32, kind="ExternalOutput")

    with tile.TileContext(nc) as tc:
        with (
            tc.tile_pool(name="singles", bufs=1) as singles,
            tc.tile_pool(name="kvq", bufs=1) as kvq,
            tc.tile_pool(name="xs", bufs=3) as xs,
            tc.tile_pool(name="xn", bufs=3) as xnp,
            tc.tile_pool(name="pt", bufs=3) as ptp,
            tc.tile_pool(name="epi", bufs=2) as epi,
            tc.tile_pool(name="outp", bufs=2) as outp,
            tc.tile_pool(name="pg", bufs=4) as pg,
            tc.tile_pool(name="sps", bufs=2, space="PSUM") as sps,
            tc.tile_pool(name="ops", bufs=1, space="PSUM") as ops,
            tc.tile_pool(name="dr", bufs=2, space="DRAM") as drp,
        ):
            # ---- load constants ----
            wT_sb = singles.tile([128, 2, 3 * C], F32, tag="wT")
            nc.sync.dma_start(out=wT_sb[:, 0, :], in_=wT_d[0:128, :])
            nc.sync.dma_start(out=wT_sb[:, 1, :], in_=wT_d[128:256, :])
            projT_sb = singles.tile([128, 2, C], F32, tag="projT")
            nc.sync.dma_start(out=projT_sb[:, 0, :], in_=projT_d[0:128, :])
            nc.sync.dma_start(out=projT_sb[:, 1, :], in_=projT_d[128:256, :])
            pvec_sb = singles.tile([128, 8], F32, tag="pvec")
            nc.sync.dma_start(out=pvec_sb, in_=pvec_d[:, :])
            projbr_sb = singles.tile([1, C], F32, tag="projbr")
            nc.sync.dma_start(out=projbr_sb, in_=projbr_d[:, :])
            G_sb = singles.tile([128, 16], F32, tag="G")
            nc.sync.dma_start(out=G_sb, in_=G_d[:, :])
            GT_sb = singles.tile([16, 128], F32, tag="GT")
            nc.sync.dma_start(out=GT_sb, in_=GT_d[:, :])
            xq_sb = singles.tile([128, 2, NQC], F32, tag="xq")
            nc.sync.dma_start(out=xq_sb[:, 0, :], in_=xq_d[0:128, :])
            nc.sync.dma_start(out=xq_sb[:, 1, :], in_=xq_d[128:256, :])

            ones_r = singles.tile([1, 512], F32, tag="ones_r")
            nc.vector.memset(ones_r, 1.0)

            # ---- GroupNorm stats (one pass over x) ----
            stats = singles.tile([128, 2, NST, 6], F32, tag="stats")
            for cc in range(2):
                for st in range(NST):
                    xt = xs.tile([128, ST], F32, tag=f"x{cc}")
                    nc.sync.dma_start(out=xt, in_=x_d[128 * cc:128 * (cc + 1), ST * st:ST * (st + 1)])
                    nc.vector.bn_stats(out=stats[:, cc, st, :], in_=xt)

            ab_sb = singles.tile([128, 2, 2], F32, tag="ab")  # [:, cc, 0]=a, [:, cc, 1]=b
            for cc in range(2):
                mv = pg.tile([128, 2], F32, tag="mv")
                nc.vector.bn_aggr(out=mv, in_=stats[:, cc, :, :])
                st2 = pg.tile([128, 2], F32, tag="st2")  # (mean, E[x^2])
                nc.vector.tensor_copy(out=st2[:, 0:1], in_=mv[:, 0:1])
                nc.vector.tensor_tensor(out=st2[:, 1:2], in0=mv[:, 0:1], in1=mv[:, 0:1], op=ALU.mult)
                nc.vector.tensor_tensor(out=st2[:, 1:2], in0=st2[:, 1:2], in1=mv[:, 1:2], op=ALU.add)
                gps = sps.tile([128, 512], F32, tag="s")
                nc.tensor.matmul(gps[0:16, 0:2], lhsT=G_sb, rhs=st2, start=True, stop=True)
                gm = pg.tile([16, 2], F32, tag="gm")  # (mean_g, E2_g)
                nc.vector.tensor_copy(out=gm, in_=gps[0:16, 0:2])
                t16 = pg.tile([16, 1], F32, tag="t16")
                nc.vector.tensor_tensor(out=t16, in0=gm[:, 0:1], in1=gm[:, 0:1], op=ALU.mult)
                nc.vector.tensor_tensor(out=gm[:, 1:2], in0=gm[:, 1:2], in1=t16, op=ALU.subtract)
                # rstd = 1/sqrt(var+eps)
                nc.vector.tensor_scalar_add(out=gm[:, 1:2], in0=gm[:, 1:2], scalar1=EPS)
                nc.scalar.activation(out=gm[:, 1:2], in_=gm[:, 1:2], func=AF.Sqrt)
                nc.vector.reciprocal(out=gm[:, 1:2], in_=gm[:, 1:2])
                mps = sps.tile([128, 512], F32, tag="s")
                nc.tensor.matmul(mps[0:128, 0:2], lhsT=GT_sb, rhs=gm, start=True, stop=True)
                mr = pg.tile([128, 2], F32, tag="mr")  # (mean_c, rstd_c)
                nc.vector.tensor_copy(out=mr, in_=mps[0:128, 0:2])
                # a = rstd * norm_w ; b = norm_b - mean * a
                nc.vector.tensor_tensor(out=ab_sb[:, cc, 0:1], in0=mr[:, 1:2], in1=pvec_sb[:, 4 + cc:5 + cc], op=ALU.mult)
                t128 = pg.tile([128, 1], F32, tag="t128")
                nc.vector.tensor_tensor(out=t128, in0=mr[:, 0:1], in1=ab_sb[:, cc, 0:1], op=ALU.mult)
                nc.vector.tensor_tensor(out=ab_sb[:, cc, 1:2], in0=pvec_sb[:, 6 + cc:7 + cc], in1=t128, op=ALU.subtract)

            # normalized query tokens (shared by both passes)
            xnq_sb = singles.tile([128, 2, NQC], F32, tag="xnq")
            for cc in range(2):
                nc.vector.tensor_scalar(
                    out=xnq_sb[:, cc, :], in0=xq_sb[:, cc, :],
                    scalar1=ab_sb[:, cc, 0:1], scalar2=ab_sb[:, cc, 1:2],
                    op0=ALU.mult, op1=ALU.add)

            # persistent per-pass buffers
            k_sb = kvq.tile([128, SEQ], F32, tag="k")
            v_sb = kvq.tile([128, SEQ // 128, 130], F32, tag="v")
            q_sb = kvq.tile([128, NQC], F32, tag="q")
            attn_sb = singles.tile([128, 2, NQC], F32, tag="attn")

            # ones columns of v (col 64 of each head slot); survives both passes
            vview = v_sb.rearrange("p j (h c) -> p j h c", h=2)
            nc.gpsimd.memset(vview[:, :, :, 64:65], 1.0)

            for p in range(2 if (phase >= 4 and phase not in (31,32,33)) else (1 if phase >= 1 else 0)):  # pass p handles heads 2p, 2p+1
                # ---- qkv for this pass ----
                for st in range(NST):
                    s0 = ST * st
                    xn_t = xnp.tile([128, 2, ST], F32, tag="xn")
                    for cc in range(2):
                        xt = xs.tile([128, ST], F32, tag=f"x{cc}")
                        nc.sync.dma_start(out=xt, in_=x_d[128 * cc:128 * (cc + 1), s0:s0 + ST])
                        nc.vector.tensor_scalar(
                            out=xn_t[:, cc, :], in0=xt,
                            scalar1=ab_sb[:, cc, 0:1], scalar2=ab_sb[:, cc, 1:2],
                            op0=ALU.mult, op1=ALU.add)
                    # kT rows 256+128p .. 256+128p+128  (2 heads x 64)
                    kps = sps.tile([128, 512], F32, tag="s")
                    for cc in range(2):
                        nc.tensor.matmul(
                            kps, lhsT=wT_sb[:, cc, C + 128 * p:C + 128 * p + 128],
                            rhs=xn_t[:, cc, :], start=(cc == 0), stop=(cc == 1))
                    nc.vector.tensor_scalar_add(out=k_sb[:, s0:s0 + ST], in0=kps, scalar1=pvec_sb[:, 2 + p:3 + p])
                    # v cols 512+128p .. (2 heads x 64); 4 token sub-chunks
                    vps = sps.tile([128, 512], F32, tag="s")
                    for mc in range(4):
                        for cc in range(2):
                            nc.tensor.matmul(
                                vps[:, 128 * mc:128 * (mc + 1)],
                                lhsT=xn_t[:, cc, 128 * mc:128 * (mc + 1)],
                                rhs=wT_sb[:, cc, 2 * C + 128 * p:2 * C + 128 * p + 128],
                                start=(cc == 0), stop=(cc == 1))
                    nc.vector.tensor_copy(
                        out=vview[:, 4 * st:4 * st + 4, :, 0:64],
                        in_=vps.rearrange("p (j h c) -> p j h c", j=4, h=2))
                # qT for this pass
                for qt0, qtn in ((0, 512), (512, 512), (1024, 128)):
                    qps = sps.tile([128, 512], F32, tag="s")
                    for cc in range(2):
                        nc.tensor.matmul(
                            qps[:, 0:qtn], lhsT=wT_sb[:, cc, 128 * p:128 * p + 128],
                            rhs=xnq_sb[:, cc, qt0:qt0 + qtn], start=(cc == 0), stop=(cc == 1))
                    nc.vector.tensor_scalar_add(out=q_sb[:, qt0:qt0 + qtn], in0=qps[:, 0:qtn], scalar1=pvec_sb[:, p:p + 1])

                # ---- attention ----
                if phase < 2:
                    continue
                # Interleave the two heads' key-chunks in one stream: adjacent
                # K=64 QK matmuls hit disjoint PE row groups (base 0 / base 64)
                # and run concurrently. exp still batches SCHUNK chunks/ACTIVATE.
                for (q0, qn, kc0, nch) in (WINDOWS[:1] if phase < 3 else (WINDOWS[:2] if phase == 31 else ([WINDOWS[0], WINDOWS[2]] if phase in (32, 33) else WINDOWS))):
                    o_t = {hl: ops.tile([128, 512], F32, tag=f"o{hl}", name=f"o{hl}") for hl in range(2)}
                    stream = [(hl, kc0 + c) for c in range(nch) for hl in range(2)]
                    done = 0
                    while done < len(stream):
                        m = min(SCHUNK, len(stream) - done)
                        items = stream[done:done + m]
                        # each QK matmul output must start at a PSUM bank
                        # boundary (col 512*j); for qn<512 exp reads strided
                        s_ps = sps.tile([128, 3, 512], F32, tag="s")
                        for j, (hl, kc) in enumerate(items):
                            r0 = 64 * hl
                            nc.tensor.matmul(
                                s_ps[:, j, 0:qn],
                                lhsT=k_sb[r0:r0 + 64, 128 * kc:128 * (kc + 1)],
                                rhs=q_sb[r0:r0 + 64, q0:q0 + qn],
                                start=True, stop=True)
                        pt = ptp.tile([128, 3 * 512], F32, tag="p")
                        ptv = pt[:, 0:m * qn].rearrange("p (j c) -> p j c", j=m)
                        nc.scalar.activation(out=ptv, in_=s_ps[:, 0:m, 0:qn], func=AF.Exp, scale=SCALE)
                        for j, (hl, kc) in enumerate(items):
                            nc.tensor.matmul(
                                o_t[hl][0:65, 0:qn],
                                lhsT=vview[:, kc, hl, :],
                                rhs=pt[:, qn * j:qn * (j + 1)],
                                start=(kc == kc0), stop=(kc == kc0 + nch - 1))
                        done += m
                    for hl in range(2):
                        # epilogue: copy O out of PSUM, normalize, place into attn^T
                        osb = epi.tile([65, 512], F32, tag="osb")
                        nc.vector.tensor_copy(out=osb[:, 0:qn], in_=o_t[hl][0:65, 0:qn])
                        rec = epi.tile([1, 512], F32, tag="rec")
                        nc.vector.reciprocal(out=rec[0:1, 0:qn], in_=osb[64:65, 0:qn])
                        rd = drp.tile([1, 512], F32, tag="rd")
                        nc.sync.dma_start(out=rd[0:1, 0:qn], in_=rec[0:1, 0:qn])
                        recb = epi.tile([64, 512], F32, tag="recb")
                        nc.sync.dma_start(out=recb[0:64, 0:qn], in_=_bcast_part(rd[0:1, 0:qn], 64))
                        if hl == 0:
                            nc.vector.tensor_tensor(
                                out=attn_sb[0:64, p, q0:q0 + qn],
                                in0=osb[0:64, 0:qn], in1=recb[0:64, 0:qn], op=ALU.mult)
                        else:
                            at = epi.tile([64, 512], F32, tag="at")
                            nc.vector.tensor_tensor(
                                out=at[:, 0:qn], in0=osb[0:64, 0:qn], in1=recb[0:64, 0:qn], op=ALU.mult)
                            nc.sync.dma_start(out=attn_sb[64:128, p, q0:q0 + qn], in_=at[:, 0:qn])

            # ---- projection + residual ----
            if phase < 5 or phase in (31,32,33):
                # debug: dump something defined to out
                for mc in range(2):
                    ot = outp.tile([128, 512], F32, tag="ot")
                    nc.vector.tensor_copy(out=ot, in_=xq_sb[:, mc, 0:512])
                    nc.sync.dma_start(out=out_d[128 * mc:128 * (mc + 1), 0:512], in_=ot)
                    ot2 = outp.tile([128, 512], F32, tag="ot")
                    nc.vector.tensor_copy(out=ot2, in_=xq_sb[:, mc, 512:1024])
                    nc.sync.dma_start(out=out_d[128 * mc:128 * (mc + 1), 512:1024], in_=ot2)
            for qt0, qtn in (() if (phase < 5 or phase in (31,32,33)) else ((0, 512), (512, 512), (1024, 128))):
                for mc in range(2):
                    pp = sps.tile([128, 512], F32, tag="s")
                    nc.tensor.matmul(pp[:, 0:qtn], lhsT=projbr_sb[0:1, 128 * mc:128 * (mc + 1)],
                                     rhs=ones_r[0:1, 0:qtn], start=True, stop=False)
                    for cc in range(2):
                        nc.tensor.matmul(pp[:, 0:qtn], lhsT=projT_sb[:, cc, 128 * mc:128 * (mc + 1)],
                                         rhs=attn_sb[:, cc, qt0:qt0 + qtn],
                                         start=False, stop=(cc == 1))
                    ot = outp.tile([128, 512], F32, tag="ot")
                    nc.vector.tensor_tensor(out=ot[:, 0:qtn], in0=pp[:, 0:qtn],
                                            in1=xq_sb[:, mc, qt0:qt0 + qtn], op=ALU.add)
                    nc.sync.dma_start(out=out_d[128 * mc:128 * (mc + 1), qt0:qt0 + qtn], in_=ot[:, 0:qtn])

    return nc


def make_inputs(x, norm_w, norm_b, qkv_w, qkv_b, proj_w, proj_b):
    """Host-side prep: full-input numpy -> per-core in_maps."""
    x2 = np.ascontiguousarray(np.asarray(x, np.float32).reshape(C, SEQ))
    qkv_w = np.asarray(qkv_w, np.float32)
    qkv_b = np.asarray(qkv_b, np.float32)
    proj_w = np.asarray(proj_w, np.float32)
    proj_b = np.asarray(proj_b, np.float32)
    norm_w = np.asarray(norm_w, np.float32)
    norm_b = np.asarray(norm_b, np.float32)

    wT = np.ascontiguousarray(qkv_w.T)
    projT = np.ascontiguousarray(proj_w.T)
    # v-bias folds into the projection bias: proj(attn + bv) = proj(attn) + proj_w @ bv
    projbr = (proj_b + proj_w @ qkv_b[2 * C:3 * C]).reshape(1, C).astype(np.float32)
    pvec = np.stack([
        qkv_b[0:128], qkv_b[128:256],          # q bias pass0/1
        qkv_b[C:C + 128], qkv_b[C + 128:2 * C],  # k bias pass0/1
        norm_w[0:128], norm_w[128:256],
        norm_b[0:128], norm_b[128:256],
    ], axis=1).astype(np.float32)
    cidx = np.arange(128)
    gidx = np.arange(16)
    G = ((cidx[:, None] // 8) == gidx[None, :]).astype(np.float32) / 8.0
    GT = np.ascontiguousarray(G.T * 8.0)

    common = dict(x=x2, wT=wT, projT=projT, pvec=pvec, projbr=projbr, G=G, GT=GT)
    in_maps = []
    cols = []
    for i in range(NCORES):
        ci = np.concatenate([
            np.arange(512 * i, 512 * (i + 1)),
            np.arange(4096 + 512 * i, 4096 + 512 * (i + 1)),
            np.arange(8192 + 128 * i, 8192 + 128 * (i + 1)),
        ])
        cols.append(ci)
        m = dict(common)
        m["xq"] = np.ascontiguousarray(x2[:, ci])
        in_maps.append(m)
    return in_maps, cols


_NC_CACHE = {}


def kernel(x, norm_w, norm_b, qkv_w, qkv_b, proj_w, proj_b):
    from concourse.bass_utils import run_bass_kernel_spmd

    _patch_tile_drain()
    _patch_to_json_split_waits()
    in_maps, cols = make_inputs(x, norm_w, norm_b, qkv_w, qkv_b, proj_w, proj_b)
    if "nc" not in _NC_CACHE:
        _NC_CACHE["nc"] = build_nc()
    nc = _NC_CACHE["nc"]
    res = run_bass_kernel_spmd(nc, in_maps, core_ids=list(range(NCORES)))
    out = np.zeros((C, SEQ), np.float32)
    for i in range(NCORES):
        out[:, cols[i]] = res.results[i]["out"]
    return out.reshape(1, C, 96, 96)


# revision 14
# speedup vs baseline: 2.0524x; 2.0524x over previous
"""Trainium2 Bass kernel for nn_AttentionBlock (GroupNorm + windowed MHA + proj + residual).

Contract: kernel(**inputs) takes FULL unsharded inputs (as from reference.setup_inputs())
and returns the FULL output [1, 256, 96, 96] float32.

Sharding: sequence-parallel over query positions across 8 cores. Each core gets a
uniform slice of each of the 3 reference attention windows:
  W0: q[512i   : 512(i+1)]    attends kv[0    : 6144]
  W1: q[4096+512i : ...]      attends kv[2048 : 9216]
  W2: q[8192+128i : ...]      attends kv[6144 : 9216]
All 4 heads for those queries are computed locally, so the output projection and
residual are local too. Every core redundantly computes GroupNorm stats and the
full-sequence K/V (needed since every core's windows span nearly the full sequence).

On-chip dataflow ("transposed" layout, channels on partitions):
  x [256, 9216]  -> bn_stats -> group stats via small PE matmuls -> per-channel a,b
  xn = a*x + b   (DVE, streamed)
  kT [2heads*64, 9216] = Wk-slice.T-matmuls (per pass of 2 heads)
  v  [9216, 2heads*65] (65th col = ones, so PV matmul also produces softmax sums)
  q  [2heads*64, 1152]
  S^T[keys, q] = kT-chunk as lhsT, qT as rhs; exp on ScalarE (PSUM->SBUF, batched
  3 key-chunks per ACTIVATE); PV: lhsT=[v|1], rhs=exp(S^T) accumulating O^T[65, q];
  normalize by broadcast reciprocal row (DRAM-bounce partition broadcast);
  proj: projT-matmuls over assembled attn^T [256, 1152] + bias (via rank-1 matmul)
  + residual, DMA out.
"""

import numpy as np

import concourse.bass as bass
import concourse.tile as tile
from concourse import mybir
from concourse.vector_clock import ScopedClock, VectorClock

F32 = mybir.dt.float32
F32R = mybir.dt.float32r  # fp32 storage, TF32-like PE mode: ~4x matmul throughput, rel err ~1.5e-4
AF = mybir.ActivationFunctionType
ALU = mybir.AluOpType

C = 256
SEQ = 9216
NCORES = 8
HEADS = 4
D = 64
EPS = 1e-5
SCALE = 0.125  # 1/sqrt(64)
NQC = 1152  # queries per core
ST = 512  # seq tile for qkv streaming
NST = SEQ // ST  # 18
# windows: (q_off, q_len, key_chunk0, n_key_chunks)
WINDOWS = [(0, 512, 0, 48), (512, 512, 16, 56), (1024, 128, 48, 24)]
SCHUNK = 3  # key-chunks batched per exp ACTIVATE (3 PSUM banks)


def _patch_tile_drain():
    """This container's walrus rejects >1 sem wait on one sync CTRL instruction
    ("Too many sync wait commands"). Split the TileContext-exit drain's waits
    into one-wait-per-nop instructions."""
    if getattr(tile.TileContext, "_drain_split_patched", False):
        return

    def _drain_and_barrier(self, tick_clock, wait_clock):
        vc = tick_clock.global_clock
        n = len(vc)
        for p in range(n):
            t = vc[p]
            if t <= 0:
                continue
            single = VectorClock([t if i == p else 0 for i in range(n)])
            inst = self.nc.sync.nop(nofuse=True, hint="drain_split_wait")
            wait_clock.add_sem_waits(inst.ins, ScopedClock({None: single}))
        self.nc.sync.drain()
        self.nc.all_engine_barrier()
        assert self.sems is not None
        popped = self.nc._tile_sem_poison_stack.pop()
        assert popped is self._sem_poison
        self.nc.clear_and_free_semaphores(list(self.sems.allocated().values()))
        self.nc.all_engine_barrier()

    tile.TileContext._drain_and_barrier = _drain_and_barrier
    tile.TileContext._drain_split_patched = True


def _patch_to_json_split_waits():
    """This walrus build rejects instructions carrying more than one sem-wait
    ("Too many sync wait commands"). Post-process the BIR JSON: keep one wait
    on each instruction and move extras onto same-engine NoOps inserted just
    before it (identical sync semantics: the engine blocks on the nops first)."""
    if getattr(bass.Bass, "_split_waits_patched", False):
        return
    import json as _json

    orig = bass.Bass.to_json_bytes

    def to_json_bytes(self):
        d = _json.loads(orig(self))
        for fn in d["functions"]:
            for blk in fn["blocks"]:
                out = []
                changed = False
                for ins in blk["instructions"]:
                    si = ins.get("sync_info")
                    waits = (si or {}).get("on_wait") or []
                    if len(waits) > 1:
                        changed = True
                        for k, w in enumerate(waits[:-1]):
                            out.append({
                                "debug": ins.get("debug", 0),
                                "engine": ins["engine"],
                                "ins": [],
                                "name": f"{ins['name']}-w{k}",
                                "opcode": "NoOp",
                                "outs": [],
                                "sync_info": {"on_update": [], "on_wait": [w]},
                                "text_hint": "split_wait",
                            })
                        si["on_wait"] = [waits[-1]]
                    out.append(ins)
                if changed:
                    blk["instructions"] = out
        return _json.dumps(d).encode()

    bass.Bass.to_json_bytes = to_json_bytes
    bass.Bass._split_waits_patched = True


def _bcast_part(ap, n):
    """[1, m] AP -> [n, m] AP broadcasting along partitions (step 0)."""
    apl = ap.ap
    assert apl[0][1] == 1, apl
    return bass.AP(tensor=ap.tensor, offset=ap.offset, ap=[[0, n]] + [list(d) for d in apl[1:]])


def build_nc(phase=99):
    nc = bass.Bass()

    x_d = nc.dram_tensor("x", [C, SEQ], F32, kind="ExternalInput")
    xq_d = nc.dram_tensor("xq", [C, NQC], F32, kind="ExternalInput")
    wT_d = nc.dram_tensor("wT", [C, 3 * C], F32, kind="ExternalInput")
    projT_d = nc.dram_tensor("projT", [C, C], F32, kind="ExternalInput")
    pvec_d = nc.dram_tensor("pvec", [128, 8], F32, kind="ExternalInput")
    projbr_d = nc.dram_tensor("projbr", [1, C], F32, kind="ExternalInput")
    G_d = nc.dram_tensor("G", [128, 16], F32, kind="ExternalInput")
    GT_d = nc.dram_tensor("GT", [16, 128], F32, kind="ExternalInput")
    out_d = nc.dram_tensor("out", [C, NQC], F32, kind="ExternalOutput")

    with tile.TileContext(nc) as tc:
        with (
            tc.tile_pool(name="singles", bufs=1) as singles,
            tc.tile_pool(name="kvq", bufs=1) as kvq,
            tc.tile_pool(name="xs", bufs=3) as xs,
            tc.tile_pool(name="xn", bufs=3) as xnp,
            tc.tile_pool(name="pt", bufs=3) as ptp,
            tc.tile_pool(name="epi", bufs=2) as epi,
            tc.tile_pool(name="outp", bufs=2) as outp,
            tc.tile_pool(name="pg", bufs=4) as pg,
            tc.tile_pool(name="sps", bufs=2, space="PSUM") as sps,
            tc.tile_pool(name="ops", bufs=1, space="PSUM") as ops,
            tc.tile_pool(name="dr", bufs=2, space="DRAM") as drp,
        ):
            # ---- load constants ----
            wT_sb = singles.tile([128, 2, 3 * C], F32, tag="wT")
            nc.sync.dma_start(out=wT_sb[:, 0, :], in_=wT_d[0:128, :])
            nc.sync.dma_start(out=wT_sb[:, 1, :], in_=wT_d[128:256, :])
            projT_sb = singles.tile([128, 2, C], F32, tag="projT")
            nc.sync.dma_start(out=projT_sb[:, 0, :], in_=projT_d[0:128, :])
            nc.sync.dma_start(out=projT_sb[:, 1, :], in_=projT_d[128:256, :])
            pvec_sb = singles.tile([128, 8], F32, tag="pvec")
            nc.sync.dma_start(out=pvec_sb, in_=pvec_d[:, :])
            projbr_sb = singles.tile([1, C], F32, tag="projbr")
            nc.sync.dma_start(out=projbr_sb, in_=projbr_d[:, :])
            G_sb = singles.tile([128, 16], F32, tag="G")
            nc.sync.dma_start(out=G_sb, in_=G_d[:, :])
            GT_sb = singles.tile([16, 128], F32, tag="GT")
            nc.sync.dma_start(out=GT_sb, in_=GT_d[:, :])
            xq_sb = singles.tile([128, 2, NQC], F32, tag="xq")
            nc.sync.dma_start(out=xq_sb[:, 0, :], in_=xq_d[0:128, :])
            nc.sync.dma_start(out=xq_sb[:, 1, :], in_=xq_d[128:256, :])

            # float32r-rounded copies of matmul operands (PE fast mode)
            wT_r = singles.tile([128, 2, 3 * C], F32R, tag="wT_r")
            nc.vector.tensor_copy(out=wT_r[:, 0, :], in_=wT_sb[:, 0, :])
            nc.vector.tensor_copy(out=wT_r[:, 1, :], in_=wT_sb[:, 1, :])
            projT_r = singles.tile([128, 2, C], F32R, tag="projT_r")
            nc.vector.tensor_copy(out=projT_r[:, 0, :], in_=projT_sb[:, 0, :])
            nc.vector.tensor_copy(out=projT_r[:, 1, :], in_=projT_sb[:, 1, :])
            projbr_r = singles.tile([1, C], F32R, tag="projbr_r")
            nc.vector.tensor_copy(out=projbr_r, in_=projbr_sb)
            ones_f = singles.tile([1, 512], F32, tag="ones_f")
            nc.vector.memset(ones_f, 1.0)
            ones_r = singles.tile([1, 512], F32R, tag="ones_r")
            nc.vector.tensor_copy(out=ones_r, in_=ones_f)

            # ---- GroupNorm stats (one pass over x) ----
            stats = singles.tile([128, 2, NST, 6], F32, tag="stats")
            for cc in range(2):
                for st in range(NST):
                    xt = xs.tile([128, ST], F32, tag=f"x{cc}")
                    nc.sync.dma_start(out=xt, in_=x_d[128 * cc:128 * (cc + 1), ST * st:ST * (st + 1)])
                    nc.vector.bn_stats(out=stats[:, cc, st, :], in_=xt)

            ab_sb = singles.tile([128, 2, 2], F32, tag="ab")  # [:, cc, 0]=a, [:, cc, 1]=b
            for cc in range(2):
                mv = pg.tile([128, 2], F32, tag="mv")
                nc.vector.bn_aggr(out=mv, in_=stats[:, cc, :, :])
                st2 = pg.tile([128, 2], F32, tag="st2")  # (mean, E[x^2])
                nc.vector.tensor_copy(out=st2[:, 0:1], in_=mv[:, 0:1])
                nc.vector.tensor_tensor(out=st2[:, 1:2], in0=mv[:, 0:1], in1=mv[:, 0:1], op=ALU.mult)
                nc.vector.tensor_tensor(out=st2[:, 1:2], in0=st2[:, 1:2], in1=mv[:, 1:2], op=ALU.add)
                gps = sps.tile([128, 512], F32, tag="s")
                nc.tensor.matmul(gps[0:16, 0:2], lhsT=G_sb, rhs=st2, start=True, stop=True)
                gm = pg.tile([16, 2], F32, tag="gm")  # (mean_g, E2_g)
                nc.vector.tensor_copy(out=gm, in_=gps[0:16, 0:2])
                t16 = pg.tile([16, 1], F32, tag="t16")
                nc.vector.tensor_tensor(out=t16, in0=gm[:, 0:1], in1=gm[:, 0:1], op=ALU.mult)
                nc.vector.tensor_tensor(out=gm[:, 1:2], in0=gm[:, 1:2], in1=t16, op=ALU.subtract)
                # rstd = 1/sqrt(var+eps)
                nc.vector.tensor_scalar_add(out=gm[:, 1:2], in0=gm[:, 1:2], scalar1=EPS)
                nc.scalar.activation(out=gm[:, 1:2], in_=gm[:, 1:2], func=AF.Sqrt)
                nc.vector.reciprocal(out=gm[:, 1:2], in_=gm[:, 1:2])
                mps = sps.tile([128, 512], F32, tag="s")
                nc.tensor.matmul(mps[0:128, 0:2], lhsT=GT_sb, rhs=gm, start=True, stop=True)
                mr = pg.tile([128, 2], F32, tag="mr")  # (mean_c, rstd_c)
                nc.vector.tensor_copy(out=mr, in_=mps[0:128, 0:2])
                # a = rstd * norm_w ; b = norm_b - mean * a
                nc.vector.tensor_tensor(out=ab_sb[:, cc, 0:1], in0=mr[:, 1:2], in1=pvec_sb[:, 4 + cc:5 + cc], op=ALU.mult)
                t128 = pg.tile([128, 1], F32, tag="t128")
                nc.vector.tensor_tensor(out=t128, in0=mr[:, 0:1], in1=ab_sb[:, cc, 0:1], op=ALU.mult)
                nc.vector.tensor_tensor(out=ab_sb[:, cc, 1:2], in0=pvec_sb[:, 6 + cc:7 + cc], in1=t128, op=ALU.subtract)

            # normalized query tokens (shared by both passes)
            xnq_sb = singles.tile([128, 2, NQC], F32R, tag="xnq")
            for cc in range(2):
                nc.vector.tensor_scalar(
                    out=xnq_sb[:, cc, :], in0=xq_sb[:, cc, :],
                    scalar1=ab_sb[:, cc, 0:1], scalar2=ab_sb[:, cc, 1:2],
                    op0=ALU.mult, op1=ALU.add)

            # persistent per-pass buffers
            k_sb = kvq.tile([128, SEQ], F32R, tag="k")
            v_sb = kvq.tile([128, SEQ // 128, 130], F32R, tag="v")
            q_sb = kvq.tile([128, NQC], F32R, tag="q")
            attn_sb = singles.tile([128, 2, NQC], F32, tag="attn")

            # ones columns of v (col 64 of each head slot); survives both passes
            vview = v_sb.rearrange("p j (h c) -> p j h c", h=2)
            ones_c = singles.tile([128, 1], F32, tag="ones_c")
            nc.vector.memset(ones_c, 1.0)
            ones_bc = bass.AP(tensor=ones_c.tensor, offset=ones_c.offset,
                              ap=[list(ones_c.ap[0]), [0, SEQ // 128], [0, 2], [1, 1]])
            nc.vector.tensor_copy(out=vview[:, :, :, 64:65], in_=ones_bc)

            for p in range(2 if (phase >= 4 and phase not in (31,32,33)) else (1 if phase >= 1 else 0)):  # pass p handles heads 2p, 2p+1
                # ---- qkv for this pass ----
                for st in range(NST):
                    s0 = ST * st
                    xn_t = xnp.tile([128, 2, ST], F32R, tag="xn")
                    for cc in range(2):
                        xt = xs.tile([128, ST], F32, tag=f"x{cc}")
                        nc.sync.dma_start(out=xt, in_=x_d[128 * cc:128 * (cc + 1), s0:s0 + ST])
                        nc.vector.tensor_scalar(
                            out=xn_t[:, cc, :], in0=xt,
                            scalar1=ab_sb[:, cc, 0:1], scalar2=ab_sb[:, cc, 1:2],
                            op0=ALU.mult, op1=ALU.add)
                    # kT rows 256+128p .. 256+128p+128  (2 heads x 64)
                    kps = sps.tile([128, 512], F32, tag="s")
                    for cc in range(2):
                        nc.tensor.matmul(
                            kps, lhsT=wT_r[:, cc, C + 128 * p:C + 128 * p + 128],
                            rhs=xn_t[:, cc, :], start=(cc == 0), stop=(cc == 1))
                    nc.vector.tensor_scalar_add(out=k_sb[:, s0:s0 + ST], in0=kps, scalar1=pvec_sb[:, 2 + p:3 + p])
                    # v cols 512+128p .. (2 heads x 64); 4 token sub-chunks
                    vps = sps.tile([128, 512], F32, tag="s")
                    for mc in range(4):
                        for cc in range(2):
                            nc.tensor.matmul(
                                vps[:, 128 * mc:128 * (mc + 1)],
                                lhsT=xn_t[:, cc, 128 * mc:128 * (mc + 1)],
                                rhs=wT_r[:, cc, 2 * C + 128 * p:2 * C + 128 * p + 128],
                                start=(cc == 0), stop=(cc == 1))
                    nc.vector.tensor_copy(
                        out=vview[:, 4 * st:4 * st + 4, :, 0:64],
                        in_=vps.rearrange("p (j h c) -> p j h c", j=4, h=2))
                # qT for this pass
                for qt0, qtn in ((0, 512), (512, 512), (1024, 128)):
                    qps = sps.tile([128, 512], F32, tag="s")
                    for cc in range(2):
                        nc.tensor.matmul(
                            qps[:, 0:qtn], lhsT=wT_r[:, cc, 128 * p:128 * p + 128],
                            rhs=xnq_sb[:, cc, qt0:qt0 + qtn], start=(cc == 0), stop=(cc == 1))
                    nc.vector.tensor_scalar_add(out=q_sb[:, qt0:qt0 + qtn], in0=qps[:, 0:qtn], scalar1=pvec_sb[:, p:p + 1])

                # ---- attention ----
                if phase < 2:
                    continue
                # Interleave the two heads' key-chunks in one stream: adjacent
                # K=64 QK matmuls hit disjoint PE row groups (base 0 / base 64)
                # and run concurrently. exp still batches SCHUNK chunks/ACTIVATE.
                for (q0, qn, kc0, nch) in (WINDOWS[:1] if phase < 3 else (WINDOWS[:2] if phase == 31 else ([WINDOWS[0], WINDOWS[2]] if phase in (32, 33) else WINDOWS))):
                    o_t = {hl: ops.tile([128, 512], F32, tag=f"o{hl}", name=f"o{hl}") for hl in range(2)}
                    stream = [(hl, kc0 + c) for c in range(nch) for hl in range(2)]
                    done = 0
                    while done < len(stream):
                        m = min(SCHUNK, len(stream) - done)
                        items = stream[done:done + m]
                        # each QK matmul output must start at a PSUM bank
                        # boundary (col 512*j); for qn<512 exp reads strided
                        s_ps = sps.tile([128, 3, 512], F32, tag="s")
                        for j, (hl, kc) in enumerate(items):
                            r0 = 64 * hl
                            nc.tensor.matmul(
                                s_ps[:, j, 0:qn],
                                lhsT=k_sb[r0:r0 + 64, 128 * kc:128 * (kc + 1)],
                                rhs=q_sb[r0:r0 + 64, q0:q0 + qn],
                                start=True, stop=True)
                        pt = ptp.tile([128, 3 * 512], F32R, tag="p")
                        ptv = pt[:, 0:m * qn].rearrange("p (j c) -> p j c", j=m)
                        nc.scalar.activation(out=ptv, in_=s_ps[:, 0:m, 0:qn], func=AF.Exp, scale=SCALE)
                        for j, (hl, kc) in enumerate(items):
                            nc.tensor.matmul(
                                o_t[hl][0:65, 0:qn],
                                lhsT=vview[:, kc, hl, :],
                                rhs=pt[:, qn * j:qn * (j + 1)],
                                start=(kc == kc0), stop=(kc == kc0 + nch - 1))
                        done += m
                    for hl in range(2):
                        # epilogue: copy O out of PSUM, normalize, place into attn^T
                        osb = epi.tile([65, 512], F32, tag="osb")
                        nc.vector.tensor_copy(out=osb[:, 0:qn], in_=o_t[hl][0:65, 0:qn])
                        rec = epi.tile([1, 512], F32, tag="rec")
                        nc.vector.reciprocal(out=rec[0:1, 0:qn], in_=osb[64:65, 0:qn])
                        rd = drp.tile([1, 512], F32, tag="rd")
                        nc.sync.dma_start(out=rd[0:1, 0:qn], in_=rec[0:1, 0:qn])
                        recb = epi.tile([64, 512], F32, tag="recb")
                        nc.sync.dma_start(out=recb[0:64, 0:qn], in_=_bcast_part(rd[0:1, 0:qn], 64))
                        if hl == 0:
                            nc.vector.tensor_tensor(
                                out=attn_sb[0:64, p, q0:q0 + qn],
                                in0=osb[0:64, 0:qn], in1=recb[0:64, 0:qn], op=ALU.mult)
                        else:
                            at = epi.tile([64, 512], F32, tag="at")
                            nc.vector.tensor_tensor(
                                out=at[:, 0:qn], in0=osb[0:64, 0:qn], in1=recb[0:64, 0:qn], op=ALU.mult)
                            nc.sync.dma_start(out=attn_sb[64:128, p, q0:q0 + qn], in_=at[:, 0:qn])

            # rounded copy of attn for the f32r projection matmuls
            attn_r = singles.tile([128, 2, NQC], F32R, tag="attn_r")
            nc.vector.tensor_copy(out=attn_r[:, 0, :], in_=attn_sb[:, 0, :])
            nc.vector.tensor_copy(out=attn_r[:, 1, :], in_=attn_sb[:, 1, :])

            # ---- projection + residual ----
            if phase < 5 or phase in (31,32,33):
                # debug: dump something defined to out
                for mc in range(2):
                    ot = outp.tile([128, 512], F32, tag="ot")
                    nc.vector.tensor_copy(out=ot, in_=xq_sb[:, mc, 0:512])
                    nc.sync.dma_start(out=out_d[128 * mc:128 * (mc + 1), 0:512], in_=ot)
                    ot2 = outp.tile([128, 512], F32, tag="ot")
                    nc.vector.tensor_copy(out=ot2, in_=xq_sb[:, mc, 512:1024])
                    nc.sync.dma_start(out=out_d[128 * mc:128 * (mc + 1), 512:1024], in_=ot2)
            for qt0, qtn in (() if (phase < 5 or phase in (31,32,33)) else ((0, 512), (512, 512), (1024, 128))):
                for mc in range(2):
                    pp = sps.tile([128, 512], F32, tag="s")
                    nc.tensor.matmul(pp[:, 0:qtn], lhsT=projbr_r[0:1, 128 * mc:128 * (mc + 1)],
                                     rhs=ones_r[0:1, 0:qtn], start=True, stop=False)
                    for cc in range(2):
                        nc.tensor.matmul(pp[:, 0:qtn], lhsT=projT_r[:, cc, 128 * mc:128 * (mc + 1)],
                                         rhs=attn_r[:, cc, qt0:qt0 + qtn],
                                         start=False, stop=(cc == 1))
                    ot = outp.tile([128, 512], F32, tag="ot")
                    nc.vector.tensor_tensor(out=ot[:, 0:qtn], in0=pp[:, 0:qtn],
                                            in1=xq_sb[:, mc, qt0:qt0 + qtn], op=ALU.add)
                    nc.sync.dma_start(out=out_d[128 * mc:128 * (mc + 1), qt0:qt0 + qtn], in_=ot[:, 0:qtn])

    return nc


def make_inputs(x, norm_w, norm_b, qkv_w, qkv_b, proj_w, proj_b):
    """Host-side prep: full-input numpy -> per-core in_maps."""
    x2 = np.ascontiguousarray(np.asarray(x, np.float32).reshape(C, SEQ))
    qkv_w = np.asarray(qkv_w, np.float32)
    qkv_b = np.asarray(qkv_b, np.float32)
    proj_w = np.asarray(proj_w, np.float32)
    proj_b = np.asarray(proj_b, np.float32)
    norm_w = np.asarray(norm_w, np.float32)
    norm_b = np.asarray(norm_b, np.float32)

    wT = np.ascontiguousarray(qkv_w.T)
    projT = np.ascontiguousarray(proj_w.T)
    # v-bias folds into the projection bias: proj(attn + bv) = proj(attn) + proj_w @ bv
    projbr = (proj_b + proj_w @ qkv_b[2 * C:3 * C]).reshape(1, C).astype(np.float32)
    pvec = np.stack([
        qkv_b[0:128], qkv_b[128:256],          # q bias pass0/1
        qkv_b[C:C + 128], qkv_b[C + 128:2 * C],  # k bias pass0/1
        norm_w[0:128], norm_w[128:256],
        norm_b[0:128], norm_b[128:256],
    ], axis=1).astype(np.float32)
    cidx = np.arange(128)
    gidx = np.arange(16)
    G = ((cidx[:, None] // 8) == gidx[None, :]).astype(np.float32) / 8.0
    GT = np.ascontiguousarray(G.T * 8.0)

    common = dict(x=x2, wT=wT, projT=projT, pvec=pvec, projbr=projbr, G=G, GT=GT)
    in_maps = []
    cols = []
    for i in range(NCORES):
        ci = np.concatenate([
            np.arange(512 * i, 512 * (i + 1)),
            np.arange(4096 + 512 * i, 4096 + 512 * (i + 1)),
            np.arange(8192 + 128 * i, 8192 + 128 * (i + 1)),
        ])
        cols.append(ci)
        m = dict(common)
        m["xq"] = np.ascontiguousarray(x2[:, ci])
        in_maps.append(m)
    return in_maps, cols


_NC_CACHE = {}


def kernel(x, norm_w, norm_b, qkv_w, qkv_b, proj_w, proj_b):
    from concourse.bass_utils import run_bass_kernel_spmd

    _patch_tile_drain()
    _patch_to_json_split_waits()
    in_maps, cols = make_inputs(x, norm_w, norm_b, qkv_w, qkv_b, proj_w, proj_b)
    if "nc" not in _NC_CACHE:
        _NC_CACHE["nc"] = build_nc()
    nc = _NC_CACHE["nc"]
    res = run_bass_kernel_spmd(nc, in_maps, core_ids=list(range(NCORES)))
    out = np.zeros((C, SEQ), np.float32)
    for i in range(NCORES):
        out[:, cols[i]] = res.results[i]["out"]
    return out.reshape(1, C, 96, 96)
